# revision 14
# baseline (speedup 1.0000x reference)
# Trainium2 Bass kernel for nn_Member_Aggregator (GNN attention aggregation).
#
# Math (per edge e with node n = segment(e), 32 edges/node):
#   e_u   = u2e[neigh_idx]                          [E, 64]
#   g_rep = g2e[nodes][seg]                         [E, 64]
#   h1    = relu(e_u @ W1a.T + g_rep @ W1b.T + b1)  [E, 64]   (att1_w = [W1a | W1b])
#   h2    = relu(h1 @ W2.T + b2)                    [E, 64]
#   lg    = h2 @ w3.T (+ b3, dropped: softmax-invariant)
#   att   = segment_softmax(lg); out[n] = sum att * e_u        [N, 64]
#
# Sharding: 5000 contiguous nodes per core (x8), tables+weights replicated.
#
# Per-core layout ("stacked" feature-major): nodes padded to 5120 = 40 blocks
# x 128 nodes. Block = 4 tiles x 1024 edges. A tile pairs nodes {16t..16t+15}
# (top, SBUF partitions 0..63) with {64+16t..} (bottom, partitions 64..127),
# so every [128, 512] activation column holds one top edge + one bottom edge
# and all matmuls use block-diagonal weights at full 128-partition width.
# Stacked column x of tile t: node-slot j = x//32, neighbor k = x%32.
#
# Gather: per-chunk SWDGE indirect DMA ([128, 1] offsets, 128 u2e rows per
# instruction — the only offset shape walrus lowers correctly; dma_gather's
# Q7 software path costs ~12ns/idx and is net slower, and multi-offset
# indirect APs mis-lower on HW). Edge index tables are preloaded once on the
# Sync engine, then each tile issues 8 gathers + 4 TensorE transposes into
# the stacked layout.
#
# Per-edge q = g_rep @ W1b.T + b1 is folded into mm1 as extra contraction rows
# (lhsT = transposed per-node q, rhs = constant node-indicator). The segment
# softmax runs on the [8, 512] logits tile with stride-0 broadcast views (no
# rearrange DMAs, unlike the original which burned ~16 Pool-engine DMAs per
# block); the attention row-broadcast onto 128 partitions is a select-matmul.
# All non-cast loads/stores ride the Sync engine (HWDGE), keeping the Pool
# engine (the bottleneck) for gathers only.

import os
import sys

import numpy as np

for _p in ("/opt/trn_rl_repo",):
    if _p not in sys.path:
        sys.path.insert(0, _p)

N_NODES = 40000
DEG = 32
D = 64
NUM_USERS = 100000
NUM_GROUPS = 50000
N_CORES = 8
NPC = N_NODES // N_CORES  # 5000 nodes per core
TPB = 4                   # tiles per block
RANGES = 4
RW = 25000                # uid range width
QUOTA = 1280              # staging slots per range per block
NSLOT = RANGES * QUOTA    # 5120 staging slots per block
IDXC = RANGES * (QUOTA // 16) + 2 * (2048 // 16)  # 576 idx cols per block

_cache = {}


def _build_program(nblk):
    """Build the SPMD per-core Bass program for `nblk` 128-node blocks."""
    import concourse.bass as bass
    import concourse.tile as tile
    from concourse import bacc, mybir
    from concourse.bass import IndirectOffsetOnAxis
    from contextlib import ExitStack

    f32 = mybir.dt.float32
    bf16 = mybir.dt.bfloat16
    i32 = mybir.dt.int32
    i16 = mybir.dt.int16
    AF = mybir.ActivationFunctionType
    ALU = mybir.AluOpType
    AX = mybir.AxisListType

    npad = nblk * 128

    nc = bacc.Bacc("TRN2", target_bir_lowering=False, debug=False,
                   num_devices=N_CORES)

    u2e = nc.dram_tensor("u2e", [NUM_USERS, D], bf16, kind="ExternalInput").ap()
    g2e = nc.dram_tensor("g2e", [NUM_GROUPS, D], f32, kind="ExternalInput").ap()
    eidx = nc.dram_tensor("eidx", [nblk * TPB * 128, 8], i32,
                          kind="ExternalInput").ap()
    gidx = nc.dram_tensor("gidx", [nblk * 64, 2], i32,
                          kind="ExternalInput").ap()
    w1a_d = nc.dram_tensor("w1a", [128, 128], bf16, kind="ExternalInput").ap()
    w1b_d = nc.dram_tensor("w1b", [128, 128], f32, kind="ExternalInput").ap()
    w2_d = nc.dram_tensor("w2", [128, 128], bf16, kind="ExternalInput").ap()
    w3_d = nc.dram_tensor("w3q", [128, TPB * 8], bf16, kind="ExternalInput").ap()
    sel_d = nc.dram_tensor("sel4", [8, TPB * 128], bf16,
                           kind="ExternalInput").ap()
    ind_d = nc.dram_tensor("ind64", [64, TPB * 512], bf16,
                           kind="ExternalInput").ap()
    b1_d = nc.dram_tensor("b1st", [128, 1], f32, kind="ExternalInput").ap()
    b2_d = nc.dram_tensor("b2st", [128, 1], f32, kind="ExternalInput").ap()
    id_d = nc.dram_tensor("ident", [128, 128], f32, kind="ExternalInput").ap()
    idb_d = nc.dram_tensor("identb", [128, 128], bf16, kind="ExternalInput").ap()
    outd = nc.dram_tensor("out", [npad, D], f32, kind="ExternalOutput").ap()

    with tile.TileContext(nc) as tc, ExitStack() as ctx:
        cp = ctx.enter_context(tc.tile_pool(name="consts", bufs=1))

        def load_const(dram_ap, shape, tag, dt=f32):
            t = cp.tile(shape, dt, tag=tag)
            nc.sync.dma_start(t[:], dram_ap)
            return t

        w1a_t = load_const(w1a_d, [128, 128], "w1a", bf16)
        w1b_t = load_const(w1b_d, [128, 128], "w1b")
        w2_t = load_const(w2_d, [128, 128], "w2", bf16)
        w3_t = load_const(w3_d, [128, TPB * 8], "w3", bf16)
        sel_t = load_const(sel_d, [8, TPB * 128], "sel", bf16)
        ind_t = load_const(ind_d, [64, TPB * 512], "ind", bf16)
        b1_t = load_const(b1_d, [128, 1], "b1")
        b2_t = load_const(b2_d, [128, 1], "b2")
        id_t = load_const(id_d, [128, 128], "ident")
        idb_t = load_const(idb_d, [128, 128], "identb", bf16)
        ei_t = cp.tile([128, nblk * TPB * 8], i32, tag="eidx")
        nc.sync.dma_start(
            ei_t[:].rearrange("p (t c) -> p t c", t=nblk * TPB),
            eidx.rearrange("(t p) c -> p t c", p=128))
        gi_t = cp.tile([64, nblk * 2], i32, tag="gidx")
        nc.sync.dma_start(
            gi_t[:].rearrange("p (b c) -> p b c", b=nblk),
            gidx.rearrange("(b p) c -> p b c", p=64))

        gep = ctx.enter_context(tc.tile_pool(name="ge", bufs=6))
        tpps = ctx.enter_context(tc.tile_pool(name="tp", bufs=2, space="PSUM"))
        eut = ctx.enter_context(tc.tile_pool(name="eut", bufs=8))
        gq = ctx.enter_context(tc.tile_pool(name="gq", bufs=3))
        qps = ctx.enter_context(tc.tile_pool(name="qpsum", bufs=2, space="PSUM"))
        mmps = ctx.enter_context(tc.tile_pool(name="mm", bufs=2, space="PSUM"))
        hsb = ctx.enter_context(tc.tile_pool(name="h", bufs=4))
        lgps = ctx.enter_context(tc.tile_pool(name="lg", bufs=1, space="PSUM"))
        abps = ctx.enter_context(tc.tile_pool(name="attb", bufs=1, space="PSUM"))
        nm = ctx.enter_context(tc.tile_pool(name="nm", bufs=3))
        wsb_p = ctx.enter_context(tc.tile_pool(name="w", bufs=3))
        wacc_p = ctx.enter_context(tc.tile_pool(name="wacc", bufs=2))
        osb_p = ctx.enter_context(tc.tile_pool(name="osb", bufs=2))

        for b in range(nblk):
            # logits psum for the whole block: partition 2t+h = (tile t, half h)
            lg8 = lgps.tile([8, 512], f32)

            # ---- q phase: per-node q = g2e[gid] @ W1b.T + b1, transposed ----
            # gidx row p = (node p, node 64+p): gt partition p holds
            # [g(p) | g(64+p)]; its transpose is the stacked g2T.
            gt = gq.tile([64, 128], f32, tag="gt")
            for c in range(2):
                nc.gpsimd.indirect_dma_start(
                    out=gt[:, D * c:D * (c + 1)], out_offset=None, in_=g2e,
                    in_offset=IndirectOffsetOnAxis(
                        ap=gi_t[:, 2 * b + c:2 * b + c + 1], axis=0))
            g2T = qps.tile([128, 128], f32, tag="qp")
            nc.tensor.transpose(out=g2T[:, 0:64], in_=gt[:],
                                identity=id_t[0:64, 0:64])
            g2T_sb = gq.tile([128, D], f32, tag="g2Tsb")
            nc.scalar.copy(g2T_sb[:], g2T[:, 0:64])
            qp = qps.tile([128, 128], f32, tag="qp")
            nc.tensor.matmul(qp[:, 0:64], lhsT=w1b_t[:], rhs=g2T_sb[:],
                             start=True, stop=True)
            q2T_sb = gq.tile([128, D], f32, tag="q2T")
            nc.vector.tensor_scalar_add(q2T_sb[:], qp[:, 0:64], b1_t[:, :1])
            qT2p = qps.tile([128, 128], f32, tag="qp")
            nc.tensor.transpose(out=qT2p[0:64, :], in_=q2T_sb[:], identity=id_t[:])
            qT2_sb = gq.tile([64, 128], bf16, tag="qT2")
            nc.scalar.copy(qT2_sb[:], qT2p[0:64, :])

            # ---- edge phase: per-chunk indirect gathers + PE transposes.
            # chunks interleave (top, bottom) rows per partition, so each
            # [128, 128] transpose writes the stacked layout at base 0.
            euts = []
            for t in range(TPB):
                ti = b * TPB + t
                ge = gep.tile([128, 512], bf16)
                for c in range(8):
                    nc.gpsimd.indirect_dma_start(
                        out=ge[:, D * c:D * (c + 1)],
                        out_offset=None, in_=u2e,
                        in_offset=IndirectOffsetOnAxis(
                            ap=ei_t[:, 8 * ti + c:8 * ti + c + 1], axis=0))
                tp = tpps.tile([128, 512], bf16)
                for u in range(4):
                    nc.tensor.transpose(
                        out=tp[:, 128 * u:128 * (u + 1)],
                        in_=ge[:, 128 * u:128 * (u + 1)], identity=idb_t[:])
                eut_sb = eut.tile([128, 512], bf16)
                nc.vector.tensor_copy(out=eut_sb[:], in_=tp[:])
                euts.append(eut_sb)

            # ---- per-tile MLP ----
            for t in range(TPB):
                h1p = mmps.tile([128, 512], f32, tag="mm")
                nc.tensor.matmul(h1p[:], lhsT=(w1a_t[:]),
                                 rhs=(euts[t][:]),
                                 start=True, stop=False)
                nc.tensor.matmul(h1p[:], lhsT=(qT2_sb[:]),
                                 rhs=(ind_t[:, t * 512:(t + 1) * 512]),
                                 start=False, stop=True)
                h1sb = hsb.tile([128, 512], bf16, tag="h")
                nc.scalar.activation(h1sb[:], h1p[:], AF.Relu)
                h2p = mmps.tile([128, 512], f32, tag="mm")
                nc.tensor.matmul(h2p[:], lhsT=(w2_t[:]),
                                 rhs=(h1sb[:]), start=True, stop=True)
                h2sb = hsb.tile([128, 512], bf16, tag="h")
                nc.scalar.activation(h2sb[:], h2p[:], AF.Relu, bias=b2_t[:, :1])
                nc.tensor.matmul(lg8[:], lhsT=(w3_t[:, 8 * t:8 * (t + 1)]),
                                 rhs=(h2sb[:]), start=(t == 0),
                                 stop=(t == TPB - 1))

            # ---- softmax over each node's 32 edges, on [8, 512] views ----
            negmax = nm.tile([8, 16], f32, tag="negmax")
            nc.vector.tensor_reduce(
                out=negmax[:], op=ALU.max, negate=True, axis=AX.X,
                in_=lg8[:].rearrange("p (j k) -> p j k", j=16))
            lgc = nm.tile([8, 512], f32, tag="lgc")
            nc.vector.tensor_tensor(
                out=lgc[:].rearrange("p (j k) -> p j k", j=16),
                in0=lg8[:].rearrange("p (j k) -> p j k", j=16),
                in1=negmax[:].unsqueeze(2).to_broadcast([8, 16, 32]),
                op=ALU.add)
            ex = nm.tile([8, 512], f32, tag="ex")
            nc.scalar.activation(ex[:], lgc[:], AF.Exp)
            sume = nm.tile([8, 16], f32, tag="sume")
            nc.vector.tensor_reduce(
                out=sume[:], op=ALU.add, axis=AX.X,
                in_=ex[:].rearrange("p (j k) -> p j k", j=16))
            rinv = nm.tile([8, 16], f32, tag="rinv")
            nc.vector.reciprocal(rinv[:], sume[:])
            att8 = nm.tile([8, 512], bf16, tag="att8")
            nc.vector.tensor_tensor(
                out=att8[:].rearrange("p (j k) -> p j k", j=16),
                in0=ex[:].rearrange("p (j k) -> p j k", j=16),
                in1=rinv[:].unsqueeze(2).to_broadcast([8, 16, 32]),
                op=ALU.mult)

            # ---- weighted aggregation ----
            wacc = wacc_p.tile([128, D], f32)
            for t in range(TPB):
                ab = abps.tile([128, 512], f32)
                nc.tensor.matmul(ab[:], lhsT=sel_t[:, 128 * t:128 * (t + 1)],
                                 rhs=att8[:], start=True, stop=True)
                wt = wsb_p.tile([128, 512], f32)
                nc.vector.tensor_tensor(out=wt[:], in0=euts[t][:],
                                        in1=ab[:], op=ALU.mult)
                nc.vector.tensor_reduce(
                    out=wacc[:, 16 * t:16 * (t + 1)],
                    in_=wt[:].rearrange("p (j k) -> p j k", j=16),
                    axis=AX.X, op=ALU.add)
            outp = qps.tile([128, 128], f32, tag="qp")
            nc.tensor.transpose(out=outp[0:64, :], in_=wacc[:], identity=id_t[:])
            osb = osb_p.tile([64, 128], f32)
            nc.scalar.copy(osb[:], outp[0:64, :])
            nc.sync.dma_start(
                outd[b * 128:(b + 1) * 128, :]
                    .rearrange("(pair n) d -> n pair d", pair=2),
                osb[:].rearrange("n (pair d) -> n pair d", pair=2))

    nc.compile()
    return nc


def _wrap16(v):
    """idx position i -> [i % 16, i // 16], replicated to 128 partitions."""
    n = len(v)
    t = np.asarray(v, np.int16).reshape(n // 16, 16).T
    return np.tile(t, (8, 1))


def _prep_host(nodes, neigh_idx, att1_w, att1_b, att2_w, att2_b, att3_w,
               nblk_per_core):
    """Shard + reorder indices, build constant tensors. Returns per-core maps
    (without the shared tables)."""
    npad = nblk_per_core * 128
    npc = min(NPC, npad)
    nodes = np.asarray(nodes).astype(np.int32)
    neigh = np.asarray(neigh_idx).astype(np.int64).reshape(-1, DEG)

    consts = {}
    att1_w = np.asarray(att1_w, np.float32)
    w1aT = att1_w[:, :D].T.copy()
    w1bT = att1_w[:, D:].T.copy()
    w2T = np.asarray(att2_w, np.float32).T.copy()

    def blockdiag(m):
        z = np.zeros((128, 128), np.float32)
        z[:64, :64] = m
        z[64:, 64:] = m
        return z

    import ml_dtypes
    bf = ml_dtypes.bfloat16
    consts["w1a"] = blockdiag(w1aT).astype(bf)
    consts["w1b"] = blockdiag(w1bT)
    consts["w2"] = blockdiag(w2T).astype(bf)
    # w3q[:, t*8 + 2t + h] = w3 half-h; tile t's mm3 writes lg8 rows 2t, 2t+1
    w3q = np.zeros((128, TPB, 8), np.float32)
    w3row = np.asarray(att3_w, np.float32)[0]
    for t in range(TPB):
        w3q[:64, t, 2 * t] = w3row
        w3q[64:, t, 2 * t + 1] = w3row
    consts["w3q"] = w3q.reshape(128, TPB * 8).astype(bf)
    # sel4[r, t*128 + m] = 1 iff r == 2t + (m >= 64): ab matmul broadcasts
    # att8 row 2t+h onto the 128 stacked feature partitions of tile t.
    sel4 = np.zeros((8, TPB, 128), np.float32)
    for t in range(TPB):
        sel4[2 * t, t, :64] = 1.0
        sel4[2 * t + 1, t, 64:] = 1.0
    consts["sel4"] = sel4.reshape(8, TPB * 128).astype(bf)
    # ind64[j, t*512 + e] = 1 iff j == 16t + e//32 (mm1b scatters per-node q)
    ind64 = np.zeros((64, TPB * 512), np.float32)
    for t in range(TPB):
        ind64[16 * t:16 * (t + 1), 512 * t:512 * (t + 1)] = np.repeat(
            np.eye(16, dtype=np.float32), 32, axis=1)
    consts["ind64"] = ind64.astype(bf)
    consts["b1st"] = np.tile(np.asarray(att1_b, np.float32), 2)[:, None].copy()
    consts["b2st"] = np.tile(np.asarray(att2_b, np.float32), 2)[:, None].copy()
    consts["ident"] = np.eye(128, dtype=np.float32)
    consts["identb"] = np.eye(128, dtype=np.float32).astype(bf)

    ncores = len(nodes) // npc if len(nodes) >= npc else 1
    per_core = []
    for c in range(ncores):
        n0 = c * npc
        # pad-node uids spread uniformly so no uid range overflows its quota
        nix = (np.arange(npad * DEG, dtype=np.int64).reshape(npad, DEG)
               * 9973) % NUM_USERS
        nix[:npc] = neigh[n0:n0 + npc]
        gid = np.zeros(npad, np.int32)
        gid[:npc] = nodes[n0:n0 + npc]

        # [b, n_local(128), k] -> [b, h, t, j, k] -> [b, t, h, j, k]
        a = nix.astype(np.int32).reshape(
            nblk_per_core, 2, TPB, 16, DEG).transpose(0, 2, 1, 3, 4)
        # flat x = 512h + 128u + p; gather chunk order interleaves (top,
        # bottom): eidx[.., p, 2u + h] = edge (h, u, p)
        a = a.reshape(nblk_per_core, TPB, 2, 4, 128).transpose(0, 1, 4, 3, 2)
        m = dict(consts)
        m["eidx"] = np.ascontiguousarray(
            a.reshape(nblk_per_core * TPB * 128, 8))
        # gidx row (b, p) = (node p, node 64+p) of block b
        m["gidx"] = np.ascontiguousarray(
            gid.reshape(nblk_per_core, 2, 64).transpose(0, 2, 1)
               .reshape(nblk_per_core * 64, 2))
        per_core.append(m)
    return per_core


def kernel(nodes, neigh_idx, segment_ids, u2e_weight, g2e_weight,
           att1_w, att1_b, att2_w, att2_b, att3_w, att3_b):
    from concourse import bass_utils

    nblk = NPC // 128 + (1 if NPC % 128 else 0)  # 40
    key = ("prog", nblk)
    if key not in _cache:
        _cache[key] = _build_program(nblk)
    nc = _cache[key]

    import ml_dtypes
    u2e = np.ascontiguousarray(
        np.asarray(u2e_weight, np.float32).astype(ml_dtypes.bfloat16))
    g2e = np.ascontiguousarray(np.asarray(g2e_weight, np.float32))
    per_core = _prep_host(nodes, neigh_idx, att1_w, att1_b, att2_w, att2_b,
                          att3_w, nblk)
    in_maps = []
    for m in per_core:
        m = dict(m)
        m["u2e"] = u2e
        m["g2e"] = g2e
        in_maps.append(m)

    res = bass_utils.run_bass_kernel_spmd(nc, in_maps,
                                          core_ids=list(range(N_CORES)))
    outs = [np.asarray(r["out"])[:NPC] for r in res.results]
    return np.concatenate(outs, axis=0)
